# revision 1
# baseline (speedup 1.0000x reference)
"""Trainium2 Bass kernel for nn_Recon_block (block-sparse attention recon).

Math (per 48x48 block, c=31 channels, N=2304 tokens):
  x:   (c, N) block pixels
  xg = w3 @ x                      (1x1 conv -> value tensor)
  S  = x^T x                       (N, N) symmetric score matrix
  P  = exp(S / sqrt(c))
  ctx[m, :] = (P[m, :] @ xg^T) / sum_n P[m, n] * (48/78)
  out = relu(ctx) viewed as (c, 48, 48) raw buffer reinterpretation

Device computes ctxT = [xg; ones] @ P  -> (32, N) per block (row 31 = softmax
denominator); host does the divide / scale / relu / relayout, which is tiny.

Sharding: 36 independent blocks data-parallel over 8 cores (5 slots per core,
cores 4..7 have one duplicated slot whose output is discarded).
"""

import numpy as np
import ml_dtypes
from contextlib import ExitStack

import concourse.bass as bass
import concourse.tile as tile
from concourse import bacc, mybir
from concourse import bass_utils

BLK = 48
C = 31
N = BLK * BLK  # 2304
NCORES = 8
SLOTS = 5
NT = N // 128  # 18 n-tiles
SCALE = 1.0 / float(np.sqrt(C))
CTX_SCALE = BLK / (BLK + C - 1.0)  # 48/78

GROUPS = [(0, 512), (512, 512), (1024, 512), (1536, 512), (2048, 256)]
BF16 = mybir.dt.bfloat16
F32 = mybir.dt.float32

_BUILT = {}


def _build(slots=SLOTS, n_cores=NCORES):
    key = (slots, n_cores)
    if key in _BUILT:
        return _BUILT[key]
    nc = bacc.Bacc("TRN2", target_bir_lowering=False, debug=False,
                   num_devices=n_cores)
    xb = nc.dram_tensor("xb", [slots, C, N], BF16, kind="ExternalInput").ap()
    w3t = nc.dram_tensor("w3t", [C, C], BF16, kind="ExternalInput").ap()
    out = nc.dram_tensor("out", [slots, C + 1, N], F32,
                         kind="ExternalOutput").ap()

    with tile.TileContext(nc) as tc, ExitStack() as ctx:
        const_pool = ctx.enter_context(tc.tile_pool(name="const", bufs=1))
        xpool = ctx.enter_context(tc.tile_pool(name="xpool", bufs=2))
        xg_pool = ctx.enter_context(tc.tile_pool(name="xg", bufs=2))
        ppool = ctx.enter_context(tc.tile_pool(name="ppool", bufs=3))
        ctx_sb_pool = ctx.enter_context(tc.tile_pool(name="ctxsb", bufs=2))
        psum_s = ctx.enter_context(
            tc.tile_pool(name="psum_s", bufs=2, space="PSUM"))
        psum_xg = ctx.enter_context(
            tc.tile_pool(name="psum_xg", bufs=2, space="PSUM"))
        psum_ctx = ctx.enter_context(
            tc.tile_pool(name="psum_ctx", bufs=2, space="PSUM"))

        w3t_sb = const_pool.tile([C, C], BF16)
        nc.sync.dma_start(w3t_sb[:], w3t[:])

        for s in range(slots):
            x_sb = xpool.tile([C, N], BF16)
            nc.sync.dma_start(x_sb[:], xb[s])

            # xgT1[n, a, :31] = xg^T tile a; [..., 31] = 1.0 (denominator row)
            xgT1 = xg_pool.tile([128, NT, 32], BF16)
            nc.vector.memset(xgT1[:], 1.0)
            for a0 in range(0, NT, 4):
                cnt = min(4, NT - a0)
                pxg = psum_xg.tile([128, 4, C], F32)
                for t in range(cnt):
                    a = a0 + t
                    nc.tensor.matmul(
                        pxg[:, t, :],
                        lhsT=x_sb[:, a * 128:(a + 1) * 128],
                        rhs=w3t_sb[:],
                        start=True, stop=True)
                nc.vector.tensor_copy(xgT1[:, a0:a0 + cnt, 0:C],
                                      pxg[:, 0:cnt, :])

            ctxT = ctx_sb_pool.tile([C + 1, N], F32)
            for (g0, W) in GROUPS:
                pctx = psum_ctx.tile([32, 512], F32)
                pending = None  # (p_tile, around)
                for r in range(9):  # 9 rounds of 2 n-tiles
                    ps = psum_s.tile([128, 2, 512], F32)
                    for t in range(2):
                        a = 2 * r + t
                        nc.tensor.matmul(
                            ps[:, t, 0:W],
                            lhsT=x_sb[:, a * 128:(a + 1) * 128],
                            rhs=x_sb[:, g0:g0 + W],
                            start=True, stop=True)
                    p_sb = ppool.tile([128, 2, 512], BF16)
                    nc.scalar.activation(p_sb[:, :, 0:W], ps[:, :, 0:W],
                                         mybir.ActivationFunctionType.Exp,
                                         scale=SCALE)
                    if pending is not None:
                        _pv(nc, pctx, xgT1, pending, W)
                    pending = (p_sb, r)
                _pv(nc, pctx, xgT1, pending, W)
                nc.vector.tensor_copy(ctxT[:, g0:g0 + W], pctx[:, 0:W])
            nc.sync.dma_start(out[s], ctxT[:])

    nc.compile()
    _BUILT[key] = nc
    return nc


def _pv(nc, pctx, xgT1, pending, W):
    p_sb, r = pending
    for t in range(2):
        a = 2 * r + t
        nc.tensor.matmul(
            pctx[0:32, 0:W],
            lhsT=xgT1[:, a, :],
            rhs=p_sb[:, t, 0:W],
            start=(a == 0), stop=(a == NT - 1),
            skip_group_check=True)


def _conv2d_np(x, w, pad):
    """x: (ci, h, w) f32; w: (co, ci, kh, kw); zero padding `pad`."""
    ci, h, wd = x.shape
    co, _, kh, kw = w.shape
    xp = np.zeros((ci, h + 2 * pad, wd + 2 * pad), np.float32)
    xp[:, pad:pad + h, pad:pad + wd] = x
    out = np.zeros((co, h, wd), np.float32)
    for dy in range(kh):
        for dx in range(kw):
            out += np.einsum('oi,iyx->oyx', w[:, :, dy, dx],
                             xp[:, dy:dy + h, dx:dx + wd],
                             optimize=True)
    return out


def kernel(xt, w1, w2, w3, wz1):
    xt = np.asarray(xt)
    w1 = np.asarray(w1)
    w2 = np.asarray(w2)
    w3 = np.asarray(w3)
    wz1 = np.asarray(wz1)

    b, c, h_inp, w_inp = xt.shape
    assert (b, c, h_inp, w_inp) == (1, C, 256, 256)
    pad_h = (-h_inp) % BLK
    pad_w = (-w_inp) % BLK
    xpad = np.pad(xt, ((0, 0), (0, 0), (0, pad_h), (0, pad_w)), mode='reflect')
    H, W = h_inp + pad_h, w_inp + pad_w
    nh, nw = H // BLK, W // BLK
    nblk = nh * nw  # 36
    # (1, c, H, W) -> (nblk, c, 48, 48) -> (nblk, c, N)
    xb_all = xpad.reshape(c, nh, BLK, nw, BLK).transpose(1, 3, 0, 2, 4)
    xb_all = np.ascontiguousarray(xb_all).reshape(nblk, c, N)

    counts = [5, 5, 5, 5, 4, 4, 4, 4]
    starts = np.concatenate([[0], np.cumsum(counts)])[:NCORES]
    xb_bf16 = xb_all.astype(ml_dtypes.bfloat16)
    w3t = np.ascontiguousarray(w3[:, :, 0, 0].T).astype(ml_dtypes.bfloat16)

    in_maps = []
    for ci_ in range(NCORES):
        blocks = [starts[ci_] + min(s, counts[ci_] - 1) for s in range(SLOTS)]
        in_maps.append({"xb": np.ascontiguousarray(xb_bf16[blocks]),
                        "w3t": w3t})

    nc = _build()
    res = bass_utils.run_bass_kernel_spmd(
        nc, in_maps, core_ids=list(range(NCORES)))

    # Host-side epilogue: normalize, scale, relu, relayout.
    zpad = np.zeros((c, H, W), np.float32)
    use_conv = float(wz1.reshape(-1)[0]) != 0.0
    wz = np.float32(wz1.reshape(-1)[0])
    for ci_ in range(NCORES):
        o = res.results[ci_]["out"]  # [SLOTS, 32, N] f32
        for s in range(counts[ci_]):
            blk = starts[ci_] + s
            ctxT = o[s]
            denom = ctxT[C]
            ctxm = ctxT[:C] * (CTX_SCALE / denom)  # (c, N)
            zblk = np.maximum(ctxm.T, 0.0).reshape(c, BLK, BLK)
            if use_conv:
                xblk = xb_all[blk].reshape(c, BLK, BLK).astype(np.float32)
                h1 = np.maximum(_conv2d_np(xblk, w1, 1), 0.0)
                z1 = xblk + _conv2d_np(h1, w2, 1)
                zblk = wz * z1 + (1.0 - wz) * zblk
            i, j = blk // nw, blk % nw
            zpad[:, i * BLK:(i + 1) * BLK, j * BLK:(j + 1) * BLK] = zblk

    return zpad[None, :, :h_inp, :w_inp].astype(np.float32)


# revision 3
# speedup vs baseline: 2153.6543x; 2153.6543x over previous
"""Trainium2 Bass kernel for nn_Recon_block (block-sparse attention recon).

Math (per 48x48 block, c=31 channels, N=2304 tokens):
  x:   (c, N) block pixels
  xg = w3 @ x                      (1x1 conv -> value tensor)
  S  = x^T x                       (N, N) symmetric score matrix
  P  = exp(S / sqrt(c))
  ctx[m, :] = (P[m, :] @ xg^T) / sum_n P[m, n] * (48/78)
  out = relu(ctx) viewed as (c, 48, 48) raw buffer reinterpretation

Device computes ctxT = [xg; ones] @ P  -> (32, N) per block (row 31 = softmax
denominator); host does the divide / scale / relu / relayout, which is tiny.

Sharding: 36 independent blocks data-parallel over 8 cores (5 slots per core,
cores 4..7 have one duplicated slot whose output is discarded).
"""

import numpy as np
import ml_dtypes
from contextlib import ExitStack

import concourse.bass as bass
import concourse.tile as tile
from concourse import bacc, mybir
from concourse import bass_utils

BLK = 48
C = 31
N = BLK * BLK  # 2304
NCORES = 8
SLOTS = 5
NT = N // 128  # 18 n-tiles
SCALE = 1.0 / float(np.sqrt(C))
CTX_SCALE = BLK / (BLK + C - 1.0)  # 48/78

GROUPS = [(0, 512), (512, 512), (1024, 512), (1536, 512), (2048, 256)]
BF16 = mybir.dt.bfloat16
F32 = mybir.dt.float32

_BUILT = {}


def _build(slots=SLOTS, n_cores=NCORES, repeat=1):
    key = (slots, n_cores, repeat)
    if key in _BUILT:
        return _BUILT[key]
    nc = bacc.Bacc("TRN2", target_bir_lowering=False, debug=False,
                   num_devices=n_cores)
    xb = nc.dram_tensor("xb", [slots, C, N], BF16, kind="ExternalInput").ap()
    w3t = nc.dram_tensor("w3t", [C, C], BF16, kind="ExternalInput").ap()
    out = nc.dram_tensor("out", [slots, C + 1, N], F32,
                         kind="ExternalOutput").ap()

    with tile.TileContext(nc) as tc, ExitStack() as ctx:
        const_pool = ctx.enter_context(tc.tile_pool(name="const", bufs=1))
        xpool = ctx.enter_context(tc.tile_pool(name="xpool", bufs=2))
        xg_pool = ctx.enter_context(tc.tile_pool(name="xg", bufs=2))
        ppool = ctx.enter_context(tc.tile_pool(name="ppool", bufs=3))
        ctx_sb_pool = ctx.enter_context(tc.tile_pool(name="ctxsb", bufs=2))
        psum_s = ctx.enter_context(
            tc.tile_pool(name="psum_s", bufs=2, space="PSUM"))
        psum_xg = ctx.enter_context(
            tc.tile_pool(name="psum_xg", bufs=2, space="PSUM"))
        psum_ctx = ctx.enter_context(
            tc.tile_pool(name="psum_ctx", bufs=2, space="PSUM"))

        w3t_sb = const_pool.tile([C, C], BF16)
        nc.sync.dma_start(w3t_sb[:], w3t[:])

        for s in [s for _ in range(repeat) for s in range(slots)]:
            x_sb = xpool.tile([C, N], BF16)
            nc.sync.dma_start(x_sb[:], xb[s])

            # xgT1[n, a, :31] = xg^T tile a; [..., 31] = 1.0 (denominator row)
            xgT1 = xg_pool.tile([128, NT, 32], BF16)
            nc.vector.memset(xgT1[:], 1.0)
            for a0 in range(0, NT, 4):
                cnt = min(4, NT - a0)
                pxg = psum_xg.tile([128, 4, C], F32)
                for t in range(cnt):
                    a = a0 + t
                    nc.tensor.matmul(
                        pxg[:, t, :],
                        lhsT=x_sb[:, a * 128:(a + 1) * 128],
                        rhs=w3t_sb[:],
                        start=True, stop=True)
                nc.vector.tensor_copy(xgT1[:, a0:a0 + cnt, 0:C],
                                      pxg[:, 0:cnt, :])

            ctxT = ctx_sb_pool.tile([C + 1, N], F32)
            for (g0, W) in GROUPS:
                pctx = psum_ctx.tile([32, 512], F32)
                pending = None  # (p_tile, around)
                for r in range(9):  # 9 rounds of 2 n-tiles
                    ps = psum_s.tile([128, 2, 512], F32)
                    for t in range(2):
                        a = 2 * r + t
                        nc.tensor.matmul(
                            ps[:, t, 0:W],
                            lhsT=x_sb[:, a * 128:(a + 1) * 128],
                            rhs=x_sb[:, g0:g0 + W],
                            start=True, stop=True)
                    p_sb = ppool.tile([128, 2, 512], BF16)
                    nc.scalar.activation(p_sb[:, :, 0:W], ps[:, :, 0:W],
                                         mybir.ActivationFunctionType.Exp,
                                         scale=SCALE)
                    if pending is not None:
                        _pv(nc, pctx, xgT1, pending, W)
                    pending = (p_sb, r)
                _pv(nc, pctx, xgT1, pending, W)
                nc.vector.tensor_copy(ctxT[:, g0:g0 + W], pctx[:, 0:W])
            nc.sync.dma_start(out[s], ctxT[:])

    nc.compile()
    _BUILT[key] = nc
    return nc


def _pv(nc, pctx, xgT1, pending, W):
    p_sb, r = pending
    for t in range(2):
        a = 2 * r + t
        nc.tensor.matmul(
            pctx[0:32, 0:W],
            lhsT=xgT1[:, a, :],
            rhs=p_sb[:, t, 0:W],
            start=(a == 0), stop=(a == NT - 1),
            skip_group_check=True)


def _conv2d_np(x, w, pad):
    """x: (ci, h, w) f32; w: (co, ci, kh, kw); zero padding `pad`."""
    ci, h, wd = x.shape
    co, _, kh, kw = w.shape
    xp = np.zeros((ci, h + 2 * pad, wd + 2 * pad), np.float32)
    xp[:, pad:pad + h, pad:pad + wd] = x
    out = np.zeros((co, h, wd), np.float32)
    for dy in range(kh):
        for dx in range(kw):
            out += np.einsum('oi,iyx->oyx', w[:, :, dy, dx],
                             xp[:, dy:dy + h, dx:dx + wd],
                             optimize=True)
    return out


def kernel(xt, w1, w2, w3, wz1):
    xt = np.asarray(xt)
    w1 = np.asarray(w1)
    w2 = np.asarray(w2)
    w3 = np.asarray(w3)
    wz1 = np.asarray(wz1)

    b, c, h_inp, w_inp = xt.shape
    assert (b, c, h_inp, w_inp) == (1, C, 256, 256)
    pad_h = (-h_inp) % BLK
    pad_w = (-w_inp) % BLK
    xpad = np.pad(xt, ((0, 0), (0, 0), (0, pad_h), (0, pad_w)), mode='reflect')
    H, W = h_inp + pad_h, w_inp + pad_w
    nh, nw = H // BLK, W // BLK
    nblk = nh * nw  # 36
    # (1, c, H, W) -> (nblk, c, 48, 48) -> (nblk, c, N)
    xb_all = xpad.reshape(c, nh, BLK, nw, BLK).transpose(1, 3, 0, 2, 4)
    xb_all = np.ascontiguousarray(xb_all).reshape(nblk, c, N)

    counts = [5, 5, 5, 5, 4, 4, 4, 4]
    starts = np.concatenate([[0], np.cumsum(counts)])[:NCORES]
    xb_bf16 = xb_all.astype(ml_dtypes.bfloat16)
    w3t = np.ascontiguousarray(w3[:, :, 0, 0].T).astype(ml_dtypes.bfloat16)

    in_maps = []
    for ci_ in range(NCORES):
        blocks = [starts[ci_] + min(s, counts[ci_] - 1) for s in range(SLOTS)]
        in_maps.append({"xb": np.ascontiguousarray(xb_bf16[blocks]),
                        "w3t": w3t})

    nc = _build()
    res = bass_utils.run_bass_kernel_spmd(
        nc, in_maps, core_ids=list(range(NCORES)))

    # Host-side epilogue: normalize, scale, relu, relayout.
    zpad = np.zeros((c, H, W), np.float32)
    use_conv = float(wz1.reshape(-1)[0]) != 0.0
    wz = np.float32(wz1.reshape(-1)[0])
    for ci_ in range(NCORES):
        o = res.results[ci_]["out"]  # [SLOTS, 32, N] f32
        for s in range(counts[ci_]):
            blk = starts[ci_] + s
            ctxT = o[s]
            denom = ctxT[C]
            ctxm = ctxT[:C] * (CTX_SCALE / denom)  # (c, N)
            zblk = np.maximum(ctxm.T, 0.0).reshape(c, BLK, BLK)
            if use_conv:
                xblk = xb_all[blk].reshape(c, BLK, BLK).astype(np.float32)
                h1 = np.maximum(_conv2d_np(xblk, w1, 1), 0.0)
                z1 = xblk + _conv2d_np(h1, w2, 1)
                zblk = wz * z1 + (1.0 - wz) * zblk
            i, j = blk // nw, blk % nw
            zpad[:, i * BLK:(i + 1) * BLK, j * BLK:(j + 1) * BLK] = zblk

    return zpad[None, :, :h_inp, :w_inp].astype(np.float32)
